# revision 1
# baseline (speedup 1.0000x reference)
"""Trainium2 Bass kernel for nn_MultiHeadAttention_52304111731071.

Sharding: 8 cores = 4 batches x 2 head-groups (tensor parallel over heads).
Each core computes q/k/v projections for its 512 channels (8 heads), partial
RoPE, full attention for its heads, and a partial O-projection; the host sums
the two partials per batch.

Layouts on device (per core):
  q_sb, k_sb : (128p, 4, 1024)  channel-on-partition, head pair per subtile
  vT_sb      : (128p, 8, 520)   time-on-partition, per-head 65 cols (64 v + 1 ones)
  scores^T   : psum (tk=128p, tq=512) -> exp -> SBUF
  pv         : psum (65p, 512) rows 0:64 = head out (d, tq), row 64 = softmax denom
  out_blk    : (128p, 4, 1024)  channel-on-partition -> O projection
"""

import sys

sys.path.insert(0, "/opt/trn_rl_repo")

import numpy as np

import concourse.bass as bass  # noqa: F401
import concourse.bacc as bacc
import concourse.mybir as mybir
import concourse.tile as tile

B, C, T, H = 4, 1024, 1024, 16
DH = 64
D_ROPE = 32
ROPE_BASE = 10000.0
P = 128
N_CORES = 8
HL = 8  # heads per core
CL = 512  # channels per core
KC = 8  # contraction subtiles (1024/128)
FP32 = mybir.dt.float32
SCALE = 1.0 / 8.0  # 1/sqrt(DH)
F32R = mybir.dt.float32r


def _build_program(repeat=1):
    nc = bacc.Bacc("TRN2", target_bir_lowering=False, debug=False)

    x_d = nc.dram_tensor("x_b", [C, T], F32R, kind="ExternalInput")
    c_d = nc.dram_tensor("c_b", [C, T], F32R, kind="ExternalInput")
    qwT_d = nc.dram_tensor("qwT", [C, CL], F32R, kind="ExternalInput")
    kwT_d = nc.dram_tensor("kwT", [C, CL], F32R, kind="ExternalInput")
    vwT_d = nc.dram_tensor("vwT", [C, CL], F32R, kind="ExternalInput")
    owT_d = nc.dram_tensor("owT", [CL, C], F32R, kind="ExternalInput")
    qb_d = nc.dram_tensor("qb", [CL], FP32, kind="ExternalInput")
    kb_d = nc.dram_tensor("kb", [CL], FP32, kind="ExternalInput")
    ob_d = nc.dram_tensor("ob", [C], FP32, kind="ExternalInput")
    cos_d = nc.dram_tensor("cosr", [P, T], FP32, kind="ExternalInput")
    sin_d = nc.dram_tensor("sins", [P, T], FP32, kind="ExternalInput")
    y_d = nc.dram_tensor("y", [C, T], FP32, kind="ExternalOutput")

    with tile.TileContext(nc) as tc:
      for _rep in range(repeat):
        with (
            tc.tile_pool(name="wq", bufs=1) as wq_p,
            tc.tile_pool(name="wk", bufs=1) as wk_p,
            tc.tile_pool(name="wv", bufs=1) as wv_p,
            tc.tile_pool(name="acts", bufs=1) as acts,
            tc.tile_pool(name="consts", bufs=1) as consts,
            tc.tile_pool(name="stream", bufs=3) as stream,
            tc.tile_pool(name="shift", bufs=2) as shift_p,
            tc.tile_pool(name="exp", bufs=5) as exp_p,
            tc.tile_pool(name="small", bufs=2) as small_p,
            tc.tile_pool(name="ysb", bufs=2) as y_p,
        ):
            # ---- big DMAs first: wk, c chunks, wv (K/V start earliest) ----
            wk = wk_p.tile([P, KC, CL], F32R)
            nc.sync.dma_start(wk[:], kwT_d.ap().rearrange("(ko p) m -> p ko m", p=P))
            cts = []
            for n in range(2):
                ct = stream.tile([P, KC, 512], F32R, tag="stream")
                cts.append(ct)
                nc.sync.dma_start(
                    ct[:],
                    c_d.ap().rearrange("(ko p) t -> p ko t", p=P)[
                        :, :, n * 512 : (n + 1) * 512
                    ],
                )
            wv = wv_p.tile([P, KC, CL], F32R)
            nc.sync.dma_start(wv[:], vwT_d.ap().rearrange("(ko p) m -> p ko m", p=P))

            # ---- tables / biases (needed by first epilogue) ----
            cosr = consts.tile([P, T], FP32)
            sins = consts.tile([P, T], FP32)
            nc.sync.dma_start(cosr[:], cos_d.ap())
            nc.sync.dma_start(sins[:], sin_d.ap())
            qb_sb = consts.tile([P, 4], FP32)
            kb_sb = consts.tile([P, 4], FP32)
            ob_sb = consts.tile([P, 8], FP32)
            nc.sync.dma_start(qb_sb[:], qb_d.ap().rearrange("(s p) -> p s", p=P))
            nc.sync.dma_start(kb_sb[:], kb_d.ap().rearrange("(s p) -> p s", p=P))
            nc.sync.dma_start(ob_sb[:], ob_d.ap().rearrange("(s p) -> p s", p=P))

            q_sb = acts.tile([P, 4, T], F32R)
            k_sb = acts.tile([P, 4, T], F32R)
            vT_sb = acts.tile([P, KC, HL * 65], F32R)
            out_sb = acts.tile([P, 4, T], F32R)
            # ones column per head (col 64 of each 65-col group)
            ones_f = consts.tile([1, 64], FP32)
            nc.any.memset(ones_f[:], 1.0)
            ones_r = consts.tile([1, 64], F32R)
            nc.vector.tensor_copy(ones_r[:], ones_f[:])
            ones_c = consts.tile([P, KC, 1], FP32)
            nc.any.memset(ones_c[:], 1.0)
            for j in range(HL):
                nc.vector.tensor_copy(
                    vT_sb[:, :, j * 65 + 64 : j * 65 + 65], ones_c[:]
                )

            def proj_epilogue_rope(dst, psum, bias_col, n):
                """dst (128,512) slice of q/k subtile: bias + partial RoPE.

                tmp = psum + bias; shift DMAs read tmp while dst = tmp*cosr
                computes in parallel (no WAR on tmp). cosr is 1.0 / sins 0.0
                on non-rope rows; sh mirrors tmp there so sh*sins = 0.
                """
                tmp = shift_p.tile([P, 512], F32R, tag="prj")
                nc.vector.tensor_scalar_add(tmp[:], psum, bias_col)
                sh = shift_p.tile([P, 512], F32R, tag="sh")
                for base in (0, 64):
                    nc.scalar.dma_start(
                        sh[base : base + 16, :], tmp[base + 16 : base + 32, :]
                    )
                    nc.scalar.dma_start(
                        sh[base + 16 : base + 32, :], tmp[base : base + 16, :]
                    )
                    nc.scalar.dma_start(
                        sh[base + 32 : base + 64, :], tmp[base + 32 : base + 64, :]
                    )
                ncol = slice(n * 512, (n + 1) * 512)
                nc.vector.tensor_tensor(
                    dst, tmp[:], cosr[:, ncol], mybir.AluOpType.mult
                )
                nc.vector.tensor_tensor(
                    sh[:, :], sh[:, :], sins[:, ncol], mybir.AluOpType.mult
                )
                nc.vector.tensor_tensor(dst, dst, sh[:, :], mybir.AluOpType.add)

            with (
                tc.tile_pool(name="psp", bufs=5, space="PSUM") as psp,
                tc.tile_pool(name="op", bufs=3, space="PSUM") as op,
            ):
                # ---- K projection + V^T projection interleaved per c-half ----
                def v_quarter(mt):
                    ctile = cts[mt // 4]
                    toff = (mt % 4) * P
                    ps = psp.tile([P, 512], FP32, tag="ps", name="psv")
                    for kc in range(KC):
                        nc.tensor.matmul(
                            ps[:],
                            ctile[:, kc, toff : toff + P],
                            wv[:, kc, :],
                            start=(kc == 0),
                            stop=(kc == KC - 1),
                        )
                    for j in range(HL):
                        nc.scalar.copy(
                            vT_sb[:, mt, j * 65 : j * 65 + 64],
                            ps[:, j * 64 : (j + 1) * 64],
                        )

                for n in range(2):
                    for sub in range(4):
                        ps = psp.tile([P, 512], FP32, tag="ps")
                        for kc in range(KC):
                            nc.tensor.matmul(
                                ps[:],
                                wk[:, kc, sub * P : (sub + 1) * P],
                                cts[n][:, kc, :],
                                start=(kc == 0),
                                stop=(kc == KC - 1),
                            )
                        proj_epilogue_rope(
                            k_sb[:, sub, n * 512 : (n + 1) * 512],
                            ps[:],
                            kb_sb[:, sub : sub + 1],
                            n,
                        )
                    for mt in range(4 * n, 4 * n + 4):
                        v_quarter(mt)

                # ---- Q projection (all subtiles) ----
                wq = wq_p.tile([P, KC, CL], F32R, tag="wqo")
                nc.sync.dma_start(
                    wq[:], qwT_d.ap().rearrange("(ko p) m -> p ko m", p=P)
                )
                xt = []
                for n in range(2):
                    t_ = stream.tile([P, KC, 512], F32R, tag="stream")
                    nc.sync.dma_start(
                        t_[:],
                        x_d.ap().rearrange("(ko p) t -> p ko t", p=P)[
                            :, :, n * 512 : (n + 1) * 512
                        ],
                    )
                    xt.append(t_)
                for sub in range(4):
                    for n in range(2):
                        ps = psp.tile([P, 512], FP32, tag="ps")
                        for kc in range(KC):
                            nc.tensor.matmul(
                                ps[:],
                                wq[:, kc, sub * P : (sub + 1) * P],
                                xt[n][:, kc, :],
                                start=(kc == 0),
                                stop=(kc == KC - 1),
                            )
                        proj_epilogue_rope(
                            q_sb[:, sub, n * 512 : (n + 1) * 512],
                            ps[:],
                            qb_sb[:, sub : sub + 1],
                            n,
                        )

                # ---- attention (n-major) + O projection per n-half ----
                wo = wq_p.tile([P, 4, T], F32R, tag="wqo")
                nc.sync.dma_start(
                    wo[:], owT_d.ap().rearrange("(ko p) m -> p ko m", p=P)
                )
                for n in range(2):
                    ncol = slice(n * 512, (n + 1) * 512)
                    for sub in range(4):
                        po = [
                            op.tile([P, 512], FP32, name=f"po{h_}", tag="po")
                            for h_ in range(2)
                        ]
                        for tk in range(KC):
                            ex = []
                            for half in range(2):
                                hb = half * 64
                                ps = psp.tile([P, 512], FP32, tag="ps")
                                nc.tensor.matmul(
                                    ps[:],
                                    k_sb[hb : hb + 64, sub, tk * P : (tk + 1) * P],
                                    q_sb[hb : hb + 64, sub, ncol],
                                    start=True,
                                    stop=True,
                                    tile_position=(hb, 0),
                                )
                                e = exp_p.tile([P, 512], F32R)
                                nc.scalar.activation(
                                    e[:],
                                    ps[:],
                                    mybir.ActivationFunctionType.Exp,
                                    scale=SCALE,
                                )
                                ex.append(e)
                            for half in range(2):
                                # local heads in subtile sub: (2*sub, 2*sub+1)
                                jcol = (2 * sub + half) * 65
                                nc.tensor.matmul(
                                    po[half][0:65, :],
                                    vT_sb[:, tk, jcol : jcol + 65],
                                    ex[half][:],
                                    start=(tk == 0),
                                    stop=(tk == KC - 1),
                                )
                        for half in range(2):
                            rcp0 = small_p.tile([1, 512], FP32, tag="rcp0")
                            nc.scalar.copy(rcp0[:], po[half][64:65, :])
                            rcp = small_p.tile([1, 512], F32R, tag="rcp")
                            with nc.allow_low_precision(reason="f32r bcast"):
                                nc.vector.reciprocal(rcp[:], rcp0[:])
                            pb = psp.tile([P, 512], FP32, tag="ps", name="pb")
                            nc.tensor.matmul(
                                pb[0:64, :], ones_r[:], rcp[:],
                                start=True, stop=True,
                            )
                            sb_b = small_p.tile([64, 512], FP32, tag="sbb")
                            nc.vector.tensor_copy(sb_b[:], pb[0:64, :])
                            if half == 0:
                                nc.vector.tensor_tensor(
                                    out_sb[0:64, sub, ncol],
                                    po[half][0:64, :],
                                    sb_b[:],
                                    mybir.AluOpType.mult,
                                )
                            else:
                                tmp = small_p.tile([64, 512], F32R, tag="tmp")
                                nc.vector.tensor_tensor(
                                    tmp[:],
                                    po[half][0:64, :],
                                    sb_b[:],
                                    mybir.AluOpType.mult,
                                )
                                nc.scalar.copy(out_sb[64:128, sub, ncol], tmp[:])

                    # O projection for this n-half (overlaps next n attention)
                    for m in range(8):
                        ps = psp.tile([P, 512], FP32, tag="ps")
                        for kc in range(4):
                            nc.tensor.matmul(
                                ps[:],
                                wo[:, kc, m * P : (m + 1) * P],
                                out_sb[:, kc, ncol],
                                start=(kc == 0),
                                stop=(kc == 3),
                            )
                        ys = y_p.tile([P, 512], FP32)
                        nc.vector.tensor_scalar_add(ys[:], ps[:], ob_sb[:, m : m + 1])
                        nc.sync.dma_start(y_d.ap()[m * P : (m + 1) * P, ncol], ys[:])

    nc.compile()
    return nc


def _rope_tables():
    theta = 1.0 / (ROPE_BASE ** (np.arange(0, D_ROPE, 2, dtype=np.float32) / D_ROPE))
    ang = np.arange(T, dtype=np.float32)[:, None] * theta[None, :]  # (T, 16)
    ang2 = np.concatenate([ang, ang], axis=1)  # (T, 32)
    cos2 = np.cos(ang2).astype(np.float32)  # (T, 32)
    sin2 = np.sin(ang2).astype(np.float32)
    cosr = np.ones((P, T), np.float32)
    sins = np.zeros((P, T), np.float32)
    for base in (0, 64):
        for d in range(D_ROPE):
            cosr[base + d] = cos2[:, d]
            sins[base + d] = sin2[:, d] * (-1.0 if d < 16 else 1.0)
    return cosr, sins


def make_in_maps(x, c, q_w, q_b, kv_w, kv_b, o_w, o_b):
    x = np.asarray(x, np.float32)
    c = np.asarray(c, np.float32)
    q_w = np.asarray(q_w, np.float32)
    q_b = np.asarray(q_b, np.float32)
    kv_w = np.asarray(kv_w, np.float32)
    kv_b = np.asarray(kv_b, np.float32)
    o_w = np.asarray(o_w, np.float32)
    o_b = np.asarray(o_b, np.float32)
    cosr, sins = _rope_tables()
    in_maps = []
    for core in range(N_CORES):
        b, g = core // 2, core % 2
        ch = slice(g * CL, (g + 1) * CL)
        ob_eff = o_w[:, ch] @ kv_b[C + g * CL : C + (g + 1) * CL]
        if g == 0:
            ob_eff = ob_eff + o_b
        in_maps.append(
            {
                "x_b": np.ascontiguousarray(x[b]),
                "c_b": np.ascontiguousarray(c[b]),
                "qwT": np.ascontiguousarray(q_w[ch, :].T),
                "kwT": np.ascontiguousarray(kv_w[ch, :].T),
                "vwT": np.ascontiguousarray(kv_w[C + g * CL : C + (g + 1) * CL, :].T),
                "owT": np.ascontiguousarray(o_w[:, ch].T),
                "qb": np.ascontiguousarray(q_b[ch]),
                "kb": np.ascontiguousarray(kv_b[ch]),
                "ob": np.ascontiguousarray(ob_eff.astype(np.float32)),
                "cosr": cosr,
                "sins": sins,
            }
        )
    return in_maps


_NC = None


def _get_nc():
    global _NC
    if _NC is None:
        _NC = _build_program()
    return _NC


def kernel(x, c, q_w, q_b, kv_w, kv_b, o_w, o_b):
    from concourse.bass_utils import run_bass_kernel_spmd

    nc = _get_nc()
    in_maps = make_in_maps(x, c, q_w, q_b, kv_w, kv_b, o_w, o_b)
    res = run_bass_kernel_spmd(nc, in_maps, core_ids=list(range(N_CORES)))
    y = np.empty((B, C, T), np.float32)
    for b in range(B):
        y[b] = res.results[2 * b]["y"] + res.results[2 * b + 1]["y"]
    return y

